# revision 13
# baseline (speedup 1.0000x reference)
"""Trainium2 Bass kernel v2 for the circular-3x3-conv cellular-automaton step.

Computation (per image):
    z   = conv3x3_circular(x, Wc) ;  Wc = w1 @ w_perc  (host-fused, [96,12,3,3])
    h   = relu(z + b1)
    u   = w2 @ h + b2
    out = x + (mask > 0.5) * u

v2 changes vs v1 (the LDW-per-matmul / cold-clock redesign):
  * conv as ONE K=108 matmul per image row: partitions (dj,di,c); dj=1,2
    blocks produced by two on-chip offset copies, so the stationary weight
    [108,96] stays loaded across all 16 rows of a chunk -> no per-MM LDWEIGHTS.
  * second matmul w2p [97,32] as 4 col-group matmuls per supertile (concurrent),
    flushed one supertile late so the PE never stalls on relu.
  * relu+bias split scalar(5)/vector(3) per chunk; mask multiply per half-chunk
    on vector; one gpsimd add per chunk; out stored bf16, 48 real partitions.
  * x for the final add gathered straight from xpad with a 4D HBM DMA.
"""

import sys

if "/opt/trn_rl_repo" not in sys.path:
    sys.path.insert(0, "/opt/trn_rl_repo")

from contextlib import ExitStack

import numpy as np
import ml_dtypes

import concourse.bass as bass
import concourse.tile as tile
from concourse import mybir
from concourse.bass_utils import run_bass_kernel_spmd

B, C, H, W = 16, 12, 384, 384
CH = 96                      # hidden channels
NCORES = 8
BLOC = B // NCORES           # images per core
W2 = W + 2                   # circular-padded row length
PADH = H + 5                 # padded rows: 1 top + 4 bottom (dj spill)
CHUNK = 16                   # image rows per processing chunk
ST = 4                       # rows per supertile (one per PE column group)
NCHUNK = H // CHUNK
NST = CHUNK // ST            # supertiles per chunk
XQLEN = CHUNK * W2           # used free length per chunk
XBLEN = (CHUNK + 2) * W2     # loaded free length (halo + shift spill)
MTILES = H // ST             # 96 supertile row-groups per image
STW = NST * W                # supertile-layout free length per chunk

_BF16 = mybir.dt.bfloat16
_F32 = mybir.dt.float32


def _spill_waits(nc):
    """walrus/trn2 accepts at most ONE sync-wait per instruction; move excess
    waits onto NoOps inserted immediately before, on the same engine."""
    nspill = 0
    for bbwrap in list(nc.bb_map.values()):
        bb = bbwrap.bb
        out = []
        for inst in bb.instructions:
            si = inst.sync_info
            if si is not None and si.on_wait and len(si.on_wait) > 1:
                waits = list(si.on_wait)
                for w in waits[1:]:
                    nop = mybir.InstNoOp(
                        name=nc.get_next_instruction_name(),
                        engine=inst.engine,
                        sync_info=mybir.SyncInfo(on_wait=[w], on_update=[]),
                        bass_nofuse=True,
                    )
                    nc.register_instruction(nop)
                    out.append(nop)
                    nspill += 1
                si.on_wait = waits[:1]
            out.append(inst)
        try:
            bb.instructions = out
        except Exception:
            bb.instructions.clear()
            bb.instructions.extend(out)
    return nspill


def _build_nc():
    nc = bass.Bass()

    xpad = nc.declare_dram_parameter("xpad", [BLOC, C, PADH, W2], _BF16, isOutput=False)
    wc108 = nc.declare_dram_parameter("wc108", [108, CH], _BF16, isOutput=False)
    w2p = nc.declare_dram_parameter("w2p", [CH + 1, 32], _BF16, isOutput=False)
    b1 = nc.declare_dram_parameter("b1", [CH, 1], _F32, isOutput=False)
    m128 = nc.declare_dram_parameter("m128", [128, MTILES * W], _BF16, isOutput=False)
    xst = nc.declare_dram_parameter("xst", [BLOC, NCHUNK, 128, STW], _BF16, isOutput=False)
    out = nc.declare_dram_parameter("out", [BLOC, NCHUNK, 128, STW], _BF16, isOutput=True)

    add = mybir.AluOpType.add
    mult = mybir.AluOpType.mult
    amax = mybir.AluOpType.max
    relu = mybir.ActivationFunctionType.Relu

    with tile.TileContext(nc) as tc, ExitStack() as ctx:
        const = ctx.enter_context(tc.tile_pool(name="const", bufs=1))
        zp = ctx.enter_context(tc.tile_pool(name="z", bufs=3, space="PSUM"))
        up = ctx.enter_context(tc.tile_pool(name="u", bufs=2, space="PSUM"))

        wc_sb = const.tile([108, CH], _BF16)
        nc.scalar.dma_start(out=wc_sb, in_=wc108[:, :])
        w2p_sb = const.tile([CH + 1, 32], _BF16)
        nc.scalar.dma_start(out=w2p_sb, in_=w2p[:, :])
        b1_sb = const.tile([CH, 1], _F32)
        nc.scalar.dma_start(out=b1_sb, in_=b1[:, :])
        m128_sb = const.tile([128, MTILES * W], _BF16)
        _half = (MTILES * W) // 2
        nc.scalar.dma_start(out=m128_sb[:, 0:_half], in_=m128[:, 0:_half])
        nc.gpsimd.dma_start(out=m128_sb[:, _half:], in_=m128[:, _half:])

        # manually double-buffered tiles (stable addresses); ht holds a whole
        # chunk of hidden rows so the second stage flushes as one batch
        hts = [
            const.tile([CH + 1, CHUNK, W], _BF16, name=f"ht{i}", tag=f"ht{i}")
            for i in range(2)
        ]
        xqs = [
            const.tile([108, XBLEN], _BF16, name=f"xqt{i}", tag=f"xqt{i}")
            for i in range(5)
        ]
        xts = [
            const.tile([128, STW], _BF16, name=f"xtt{i}", tag=f"xtt{i}")
            for i in range(4)
        ]
        ots = [
            const.tile([128, STW], _BF16, name=f"ott{i}", tag=f"ott{i}")
            for i in range(2)
        ]
        ums = [
            const.tile([128, STW], _BF16, name=f"umt{i}", tag=f"umt{i}")
            for i in range(2)
        ]
        def _issue_load(cj):
            bj, chkj = chunks[cj]
            base = (bj * C * PADH + chkj * CHUNK) * W2
            for dj in range(3):
                srcj = bass.AP(
                    tensor=xpad,
                    offset=base + dj,
                    ap=[[W2, 3], [PADH * W2, C], [1, XBLEN]],
                )
                nc.sync.dma_start(
                    out=xqs[cj % 5][36 * dj : 36 * dj + 36, :], in_=srcj
                )

        def _issue_xt(cj):
            bj, chkj = chunks[cj]
            nc.gpsimd.dma_start(out=xts[cj % 4], in_=xst[bj, chkj])

        chunks = [(b, chk) for b in range(BLOC) for chk in range(NCHUNK)]
        _issue_load(0)
        _issue_xt(0)
        _issue_load(1)
        _issue_xt(1)
        _issue_load(2)

        nc.vector.memset(xqs[4], 0.0)
        for ht in hts:
            nc.vector.memset(ht, 0.0)
            nc.vector.memset(ht[CH : CH + 1, :, :], 1.0)

        # HAM warmup burst: ~9us of dense back-to-back matmuls flips the PE
        # clock gate to 8/8; steady-state gaps are too short to re-throttle.
        wbt = zp.tile([CH, 2, 512], _F32, tag="z")
        for _ in range(36):
            nc.tensor.matmul(
                wbt[:, 0, 0:W], wc_sb, xqs[4][0:108, 0:W], start=True, stop=True
            )

        # relu engine assignment per z-tile index t (0..7): 5 scalar / 3 vector
        vec_tiles = {1, 3, 5}

        chunks = [(b, chk) for b in range(BLOC) for chk in range(NCHUNK)]
        pend_sec = None  # previous chunk awaiting its last supertile flush
        pend_add = None  # previous chunk awaiting its +x add and store

        def issue_load(cj):
            if cj < len(chunks):
                _issue_load(cj)

        def issue_xt(cj):
            if cj < len(chunks):
                _issue_xt(cj)


        for ci, (b, chk) in enumerate(chunks):
            r0 = chk * CHUNK
            xq = xqs[ci % 5]
            xt = xts[ci % 4]
            ot = ots[ci % 2]
            um = ums[ci % 2]

            ht = hts[ci % 2]
            for t in range(8):
                z = zp.tile([CH, 2, 512], _F32, tag="z")
                for r2 in range(2):
                    q = 2 * t + r2
                    nc.tensor.matmul(
                        z[:, r2, 0:W],
                        wc_sb,
                        xq[0:108, q * W2 : q * W2 + W],
                        start=True,
                        stop=True,
                    )

                if t == 0:
                    issue_load(ci + 3)
                    issue_xt(ci + 2)
                if t % 2 == 1 and pend_sec is not None:
                    _fin_sec_st(nc, pend_sec, (t - 1) // 2, mult)
                    if t == 5:
                        _fin_add(nc, pend_sec, add, 0)
                    if t == 7:
                        _fin_add(nc, pend_sec, add, 1)
                        pend_sec = None

                # relu+bias -> ht rows
                if t in vec_tiles:
                    nc.vector.tensor_scalar(
                        ht[0:CH, 2 * t : 2 * t + 2, :], z[:, :, 0:W],
                        b1_sb, 0.0, add, amax,
                    )
                else:
                    nc.scalar.activation(
                        out=ht[0:CH, 2 * t : 2 * t + 2, :], in_=z[:, :, 0:W],
                        func=relu, bias=b1_sb,
                    )

            pend_sec = dict(
                b=b, chk=chk, um=um, xt=xt, ot=ot, ht=ht, chkk=chk,
                m128_sb=m128_sb, out=out, w2p_sb=w2p_sb, up=up,
            )

        for st in range(NST):
            _fin_sec_st(nc, pend_sec, st, mult)
        _fin_add(nc, pend_sec, add, 0)
        _fin_add(nc, pend_sec, add, 1)
    _spill_waits(nc)
    return nc


def _mslice(m128_sb, chk, st):
    a = (chk * NST + st) * W
    return m128_sb[:, a : a + W]


def _fin_sec_st(nc, prev, st, mult):
    """Flush one supertile of the previous chunk's second stage and mask it.
    Lagging a full chunk, its relu inputs are long since complete."""
    ht = prev["ht"]
    u = prev["up"].tile([128, 512], _F32, tag="u")
    for j in range(4):
        nc.tensor.matmul(
            u[32 * j : 32 * j + 32, 0:W],
            prev["w2p_sb"],
            ht[:, 4 * st + j, :],
            start=True,
            stop=True,
            tile_position=(0, 32 * j),
        )
    nc.vector.tensor_tensor(
        prev["um"][:, st * W : st * W + W],
        u[:, 0:W],
        _mslice(prev["m128_sb"], prev["chkk"], st),
        mult,
    )


def _fin_add(nc, prev, add, half):
    """+x add (in halves) and store of the previous chunk."""
    a = half * 2 * W
    bnd = (half + 1) * 2 * W
    nc.gpsimd.tensor_tensor(
        prev["ot"][:, a:bnd], prev["um"][:, a:bnd], prev["xt"][:, a:bnd], add
    )
    if half == 1:
        nc.gpsimd.dma_start(out=prev["out"][prev["b"], prev["chk"]], in_=prev["ot"])


_NC_CACHE = {}


def _get_nc():
    if "nc" not in _NC_CACHE:
        _NC_CACHE["nc"] = _build_nc()
    return _NC_CACHE["nc"]


def _prep_inputs(x, w_perc, w1, b1, w2, b2, mask):
    bf16 = ml_dtypes.bfloat16
    wc = np.einsum("hp,pcij->hcij", w1, w_perc).astype(np.float32)  # [96,12,3,3]
    # wc108[36*dj + 12*di + c, h] = wc[h, c, di, dj]
    wdjdic = wc.transpose(3, 2, 1, 0)  # [dj, di, c, h]
    wc108 = np.ascontiguousarray(wdjdic.reshape(108, CH)).astype(bf16)
    w2p = np.zeros((CH + 1, 32), np.float32)
    w2p[0:CH, 0:C] = w2.T
    w2p[CH, 0:C] = b2
    w2p = w2p.astype(bf16)
    b1c = np.ascontiguousarray(b1.reshape(CH, 1)).astype(np.float32)

    mbit = (mask > 0.5).astype(np.float32)
    m128 = np.zeros((128, MTILES * W), np.float32)
    for j in range(ST):
        rows = mbit[j::ST, :].reshape(MTILES * W)
        for c in range(C):
            m128[32 * j + c] = rows
    m128 = m128.astype(bf16)

    xb16 = x.astype(bf16)
    in_maps = []
    for core in range(NCORES):
        xs = np.ascontiguousarray(x[core * BLOC : (core + 1) * BLOC], np.float32)
        xsp = np.pad(
            xb16[core * BLOC : (core + 1) * BLOC],
            ((0, 0), (0, 0), (1, 4), (1, 1)),
            mode="wrap",
        )
        # supertile layout: xst[b, chk, 32*j+c, s*W+w] = x[b, c, 16*chk+4*s+j, w]
        xstb = np.zeros((BLOC, NCHUNK, ST, 32, NST, W), np.float32)
        xr = xs.reshape(BLOC, C, NCHUNK, NST, ST, W).transpose(0, 2, 4, 1, 3, 5)
        xstb[:, :, :, 0:C] = xr
        xstb = xstb.reshape(BLOC, NCHUNK, 128, STW).astype(bf16)
        in_maps.append(
            {
                "xpad": np.ascontiguousarray(xsp),
                "xst": np.ascontiguousarray(xstb),
                "wc108": wc108,
                "w2p": w2p,
                "b1": b1c,
                "m128": m128,
            }
        )
    return in_maps


def _unshard_out(core_outs):
    full = np.empty((B, C, H, W), np.float32)
    for core, o in enumerate(core_outs):
        o = np.asarray(o, np.float32).reshape(BLOC, NCHUNK, ST, 32, NST, W)
        o = o[:, :, :, 0:C]  # drop pad partitions
        o = o.transpose(0, 3, 1, 4, 2, 5)  # [b, c, chk, s, j, w]
        full[core * BLOC : (core + 1) * BLOC] = o.reshape(BLOC, C, H, W)
    return full


def kernel(x, w_perc, w1, b1, w2, b2, mask):
    x = np.asarray(x, dtype=np.float32)
    in_maps = _prep_inputs(
        x,
        np.asarray(w_perc, np.float32),
        np.asarray(w1, np.float32),
        np.asarray(b1, np.float32),
        np.asarray(w2, np.float32),
        np.asarray(b2, np.float32),
        np.asarray(mask, np.float32),
    )
    nc = _get_nc()
    res = run_bass_kernel_spmd(nc, in_maps, core_ids=list(range(NCORES)))
    return _unshard_out([r["out"] for r in res.results])


# revision 14
# speedup vs baseline: 1.6669x; 1.6669x over previous
"""Trainium2 Bass kernel v2 for the circular-3x3-conv cellular-automaton step.

Computation (per image):
    z   = conv3x3_circular(x, Wc) ;  Wc = w1 @ w_perc  (host-fused, [96,12,3,3])
    h   = relu(z + b1)
    u   = w2 @ h + b2
    out = x + (mask > 0.5) * u

v2 changes vs v1 (the LDW-per-matmul / cold-clock redesign):
  * conv as ONE K=108 matmul per image row: partitions (dj,di,c); dj=1,2
    blocks produced by two on-chip offset copies, so the stationary weight
    [108,96] stays loaded across all 16 rows of a chunk -> no per-MM LDWEIGHTS.
  * second matmul w2p [97,32] as 4 col-group matmuls per supertile (concurrent),
    flushed one supertile late so the PE never stalls on relu.
  * relu+bias split scalar(5)/vector(3) per chunk; mask multiply per half-chunk
    on vector; one gpsimd add per chunk; out stored bf16, 48 real partitions.
  * x for the final add gathered straight from xpad with a 4D HBM DMA.
"""

import sys

if "/opt/trn_rl_repo" not in sys.path:
    sys.path.insert(0, "/opt/trn_rl_repo")

from contextlib import ExitStack

import numpy as np
import ml_dtypes

import concourse.bass as bass
import concourse.tile as tile
from concourse import mybir
from concourse.bass_utils import run_bass_kernel_spmd

B, C, H, W = 16, 12, 384, 384
CH = 96                      # hidden channels
NCORES = 8
BLOC = B // NCORES           # images per core
W2 = W + 2                   # circular-padded row length
PADH = H + 5                 # padded rows: 1 top + 4 bottom (dj spill)
CHUNK = 16                   # image rows per processing chunk
ST = 4                       # rows per supertile (one per PE column group)
NCHUNK = H // CHUNK
NST = CHUNK // ST            # supertiles per chunk
XQLEN = CHUNK * W2           # used free length per chunk
XBLEN = (CHUNK + 2) * W2     # loaded free length (halo + shift spill)
MTILES = H // ST             # 96 supertile row-groups per image
STW = NST * W                # supertile-layout free length per chunk

_BF16 = mybir.dt.bfloat16
_F32 = mybir.dt.float32


def _spill_waits(nc):
    """walrus/trn2 accepts at most ONE sync-wait per instruction; move excess
    waits onto NoOps inserted immediately before, on the same engine."""
    nspill = 0
    for bbwrap in list(nc.bb_map.values()):
        bb = bbwrap.bb
        out = []
        for inst in bb.instructions:
            si = inst.sync_info
            if si is not None and si.on_wait and len(si.on_wait) > 1:
                waits = list(si.on_wait)
                for w in waits[1:]:
                    nop = mybir.InstNoOp(
                        name=nc.get_next_instruction_name(),
                        engine=inst.engine,
                        sync_info=mybir.SyncInfo(on_wait=[w], on_update=[]),
                        bass_nofuse=True,
                    )
                    nc.register_instruction(nop)
                    out.append(nop)
                    nspill += 1
                si.on_wait = waits[:1]
            out.append(inst)
        try:
            bb.instructions = out
        except Exception:
            bb.instructions.clear()
            bb.instructions.extend(out)
    return nspill


def _build_nc():
    nc = bass.Bass()

    xpad = nc.declare_dram_parameter("xpad", [BLOC, C, PADH, W2], _BF16, isOutput=False)
    wc108 = nc.declare_dram_parameter("wc108", [108, CH], _BF16, isOutput=False)
    w2p = nc.declare_dram_parameter("w2p", [CH + 1, 32], _BF16, isOutput=False)
    b1 = nc.declare_dram_parameter("b1", [CH, 1], _F32, isOutput=False)
    m128 = nc.declare_dram_parameter("m128", [128, MTILES * W], _BF16, isOutput=False)
    xst = nc.declare_dram_parameter("xst", [BLOC, NCHUNK, 128, STW], _BF16, isOutput=False)
    out = nc.declare_dram_parameter("out", [BLOC, NCHUNK, 128, STW], _BF16, isOutput=True)

    add = mybir.AluOpType.add
    mult = mybir.AluOpType.mult
    amax = mybir.AluOpType.max
    relu = mybir.ActivationFunctionType.Relu

    with tile.TileContext(nc) as tc, ExitStack() as ctx:
        const = ctx.enter_context(tc.tile_pool(name="const", bufs=1))
        zp = ctx.enter_context(tc.tile_pool(name="z", bufs=3, space="PSUM"))
        up = ctx.enter_context(tc.tile_pool(name="u", bufs=2, space="PSUM"))

        wc_sb = const.tile([108, CH], _BF16)
        nc.scalar.dma_start(out=wc_sb, in_=wc108[:, :])
        w2p_sb = const.tile([CH + 1, 32], _BF16)
        nc.scalar.dma_start(out=w2p_sb, in_=w2p[:, :])
        b1_sb = const.tile([CH, 1], _F32)
        nc.scalar.dma_start(out=b1_sb, in_=b1[:, :])
        m128_sb = const.tile([128, MTILES * W], _BF16)
        _half = (MTILES * W) // 2
        nc.scalar.dma_start(out=m128_sb[:, 0:_half], in_=m128[:, 0:_half])
        nc.gpsimd.dma_start(out=m128_sb[:, _half:], in_=m128[:, _half:])

        # manually double-buffered tiles (stable addresses); ht holds a whole
        # chunk of hidden rows so the second stage flushes as one batch
        hts = [
            const.tile([CH + 1, CHUNK, W], _BF16, name=f"ht{i}", tag=f"ht{i}")
            for i in range(2)
        ]
        xqs = [
            const.tile([108, XBLEN], _BF16, name=f"xqt{i}", tag=f"xqt{i}")
            for i in range(5)
        ]
        xts = [
            const.tile([128, STW], _BF16, name=f"xtt{i}", tag=f"xtt{i}")
            for i in range(4)
        ]
        ots = [
            const.tile([128, STW], _BF16, name=f"ott{i}", tag=f"ott{i}")
            for i in range(2)
        ]
        ums = [
            const.tile([128, STW], _BF16, name=f"umt{i}", tag=f"umt{i}")
            for i in range(2)
        ]
        def _issue_load(cj):
            bj, chkj = chunks[cj]
            srcj = bass.AP(
                tensor=xpad,
                offset=(bj * C * PADH + chkj * CHUNK) * W2,
                ap=[[W2, 3], [PADH * W2, C], [1, XBLEN]],
            )
            nc.sync.dma_start(out=xqs[cj % 5][0:36, :], in_=srcj)

        def _issue_shifts(cj):
            xqj = xqs[cj % 5]
            nc.sync.dma_start(out=xqj[36:72, 0:XQLEN], in_=xqj[0:36, 1 : 1 + XQLEN])
            nc.sync.dma_start(out=xqj[72:108, 0:XQLEN], in_=xqj[0:36, 2 : 2 + XQLEN])

        def _issue_xt(cj):
            bj, chkj = chunks[cj]
            nc.gpsimd.dma_start(out=xts[cj % 4], in_=xst[bj, chkj])

        chunks = [(b, chk) for b in range(BLOC) for chk in range(NCHUNK)]
        _issue_load(0)
        _issue_shifts(0)
        _issue_xt(0)
        _issue_load(1)
        _issue_shifts(1)
        _issue_xt(1)
        _issue_load(2)

        nc.vector.memset(xqs[4], 0.0)
        for ht in hts:
            nc.vector.memset(ht, 0.0)
            nc.vector.memset(ht[CH : CH + 1, :, :], 1.0)

        # HAM warmup burst: ~9us of dense back-to-back matmuls flips the PE
        # clock gate to 8/8; steady-state gaps are too short to re-throttle.
        wbt = zp.tile([CH, 2, 512], _F32, tag="z")
        for _ in range(36):
            nc.tensor.matmul(
                wbt[:, 0, 0:W], wc_sb, xqs[4][0:108, 0:W], start=True, stop=True
            )

        # relu engine assignment per z-tile index t (0..7): 5 scalar / 3 vector
        vec_tiles = {1, 3, 5}

        chunks = [(b, chk) for b in range(BLOC) for chk in range(NCHUNK)]
        pend_sec = None  # previous chunk awaiting its last supertile flush
        pend_add = None  # previous chunk awaiting its +x add and store

        def issue_load(cj):
            if cj < len(chunks):
                _issue_load(cj)

        def issue_xt(cj):
            if cj < len(chunks):
                _issue_shifts(cj)
                _issue_xt(cj)


        for ci, (b, chk) in enumerate(chunks):
            r0 = chk * CHUNK
            xq = xqs[ci % 5]
            xt = xts[ci % 4]
            ot = ots[ci % 2]
            um = ums[ci % 2]

            ht = hts[ci % 2]
            for t in range(8):
                z = zp.tile([CH, 2, 512], _F32, tag="z")
                for r2 in range(2):
                    q = 2 * t + r2
                    nc.tensor.matmul(
                        z[:, r2, 0:W],
                        wc_sb,
                        xq[0:108, q * W2 : q * W2 + W],
                        start=True,
                        stop=True,
                    )

                if t == 0:
                    issue_load(ci + 3)
                    issue_xt(ci + 2)
                if t % 2 == 1 and pend_sec is not None:
                    _fin_sec_st(nc, pend_sec, (t - 1) // 2, mult)
                    if t == 5:
                        _fin_add(nc, pend_sec, add, 0)
                    if t == 7:
                        _fin_add(nc, pend_sec, add, 1)
                        pend_sec = None

                # relu+bias -> ht rows
                if t in vec_tiles:
                    nc.vector.tensor_scalar(
                        ht[0:CH, 2 * t : 2 * t + 2, :], z[:, :, 0:W],
                        b1_sb, 0.0, add, amax,
                    )
                else:
                    nc.scalar.activation(
                        out=ht[0:CH, 2 * t : 2 * t + 2, :], in_=z[:, :, 0:W],
                        func=relu, bias=b1_sb,
                    )

            pend_sec = dict(
                b=b, chk=chk, um=um, xt=xt, ot=ot, ht=ht, chkk=chk,
                m128_sb=m128_sb, out=out, w2p_sb=w2p_sb, up=up,
            )

        for st in range(NST):
            _fin_sec_st(nc, pend_sec, st, mult)
        _fin_add(nc, pend_sec, add, 0)
        _fin_add(nc, pend_sec, add, 1)
    _spill_waits(nc)
    return nc


def _mslice(m128_sb, chk, st):
    a = (chk * NST + st) * W
    return m128_sb[:, a : a + W]


def _fin_sec_st(nc, prev, st, mult):
    """Flush one supertile of the previous chunk's second stage and mask it.
    Lagging a full chunk, its relu inputs are long since complete."""
    ht = prev["ht"]
    u = prev["up"].tile([128, 512], _F32, tag="u")
    for j in range(4):
        nc.tensor.matmul(
            u[32 * j : 32 * j + 32, 0:W],
            prev["w2p_sb"],
            ht[:, 4 * st + j, :],
            start=True,
            stop=True,
            tile_position=(0, 32 * j),
        )
    nc.vector.tensor_tensor(
        prev["um"][:, st * W : st * W + W],
        u[:, 0:W],
        _mslice(prev["m128_sb"], prev["chkk"], st),
        mult,
    )


def _fin_add(nc, prev, add, half):
    """+x add (in halves) and store of the previous chunk."""
    a = half * 2 * W
    bnd = (half + 1) * 2 * W
    nc.gpsimd.tensor_tensor(
        prev["ot"][:, a:bnd], prev["um"][:, a:bnd], prev["xt"][:, a:bnd], add
    )
    if half == 1:
        nc.gpsimd.dma_start(out=prev["out"][prev["b"], prev["chk"]], in_=prev["ot"])


_NC_CACHE = {}


def _get_nc():
    if "nc" not in _NC_CACHE:
        _NC_CACHE["nc"] = _build_nc()
    return _NC_CACHE["nc"]


def _prep_inputs(x, w_perc, w1, b1, w2, b2, mask):
    bf16 = ml_dtypes.bfloat16
    wc = np.einsum("hp,pcij->hcij", w1, w_perc).astype(np.float32)  # [96,12,3,3]
    # wc108[36*dj + 12*di + c, h] = wc[h, c, di, dj]
    wdjdic = wc.transpose(3, 2, 1, 0)  # [dj, di, c, h]
    wc108 = np.ascontiguousarray(wdjdic.reshape(108, CH)).astype(bf16)
    w2p = np.zeros((CH + 1, 32), np.float32)
    w2p[0:CH, 0:C] = w2.T
    w2p[CH, 0:C] = b2
    w2p = w2p.astype(bf16)
    b1c = np.ascontiguousarray(b1.reshape(CH, 1)).astype(np.float32)

    mbit = (mask > 0.5).astype(np.float32)
    m128 = np.zeros((128, MTILES * W), np.float32)
    for j in range(ST):
        rows = mbit[j::ST, :].reshape(MTILES * W)
        for c in range(C):
            m128[32 * j + c] = rows
    m128 = m128.astype(bf16)

    xb16 = x.astype(bf16)
    in_maps = []
    for core in range(NCORES):
        xs = np.ascontiguousarray(x[core * BLOC : (core + 1) * BLOC], np.float32)
        xsp = np.pad(
            xb16[core * BLOC : (core + 1) * BLOC],
            ((0, 0), (0, 0), (1, 4), (1, 1)),
            mode="wrap",
        )
        # supertile layout: xst[b, chk, 32*j+c, s*W+w] = x[b, c, 16*chk+4*s+j, w]
        xstb = np.zeros((BLOC, NCHUNK, ST, 32, NST, W), np.float32)
        xr = xs.reshape(BLOC, C, NCHUNK, NST, ST, W).transpose(0, 2, 4, 1, 3, 5)
        xstb[:, :, :, 0:C] = xr
        xstb = xstb.reshape(BLOC, NCHUNK, 128, STW).astype(bf16)
        in_maps.append(
            {
                "xpad": np.ascontiguousarray(xsp),
                "xst": np.ascontiguousarray(xstb),
                "wc108": wc108,
                "w2p": w2p,
                "b1": b1c,
                "m128": m128,
            }
        )
    return in_maps


def _unshard_out(core_outs):
    full = np.empty((B, C, H, W), np.float32)
    for core, o in enumerate(core_outs):
        o = np.asarray(o, np.float32).reshape(BLOC, NCHUNK, ST, 32, NST, W)
        o = o[:, :, :, 0:C]  # drop pad partitions
        o = o.transpose(0, 3, 1, 4, 2, 5)  # [b, c, chk, s, j, w]
        full[core * BLOC : (core + 1) * BLOC] = o.reshape(BLOC, C, H, W)
    return full


def kernel(x, w_perc, w1, b1, w2, b2, mask):
    x = np.asarray(x, dtype=np.float32)
    in_maps = _prep_inputs(
        x,
        np.asarray(w_perc, np.float32),
        np.asarray(w1, np.float32),
        np.asarray(b1, np.float32),
        np.asarray(w2, np.float32),
        np.asarray(b2, np.float32),
        np.asarray(mask, np.float32),
    )
    nc = _get_nc()
    res = run_bass_kernel_spmd(nc, in_maps, core_ids=list(range(NCORES)))
    return _unshard_out([r["out"] for r in res.results])
